# revision 7
# baseline (speedup 1.0000x reference)
"""DenseMRConv (gnn message passing) on 8 TRN2 NeuronCores via Bass/Tile.

Math (reference):
    x_j  = x[edge_index]                      # [N, K, d] gather
    diff = max_k(x_j - x_i) = max_k(x_j) - x  # max distributes over const
    out  = concat([x, diff]) @ W + b
         = x @ (W_top - W_bot) + max_k(x_j) @ W_bot + b

Strategy (v2 — dma_gather based):
  The old per-k indirect_dma_start gather serialized ~400K descriptor
  generations on the gpsimd Q7 at ~9ns each (3.5ms busy).  dma_gather
  (InstDMAGatherAnt) generates descriptors at ~0.34ns each and moves
  thousands of rows per instruction, so the gather becomes DMA/HBM-bound.

  dma_gather limits: int16 indices (so the 100K-row table is split into
  4 banks of 25000 rows, each with a -1e30 dummy row at local index 0)
  and 256B elements (so x rows are stored bf16 padded to 128 features).
  transpose=True writes gathered rows feature-major: partition=feature,
  free col=gather position.  Position g of a call lands at dst col g, so
  we lay out each group of G=4 tiles (512 node-columns) slot-major:
  position = slot*512 + col, node col j owns all slots' col j.  A node's
  K=32 edges are distributed per-bank into slots [SO_b, SO_b+S_gb) where
  S_gb = max per-node bank-b degree over the group (cross-core max so the
  SPMD instruction stream is identical on all 8 cores); unused slots
  point at the bank's dummy row (-1e30, neutral for max).  Nodes are
  permuted per-core (sorted by dominant-bank profile) to homogenize bank
  histograms within a group, which cuts slot padding from ~2.2x to ~1.4x;
  the permutation is inverted for free on the host after the run.

  A single DVE halving tree over slots then yields M^T = max_k(x_j)^T
  [128 feats, 512 nodes] directly feature-major, so the MLP is just
  3 accumulating matmuls with constant weights (no PE transposes):
      outT[o, j] = A_pad[c,o]^T.. : psum = A_pad.T@xsT + Wb_pad.T@M^T + b
  with xsT the (permuted) self-features loaded feature-major straight
  from DRAM.  outT [64, nodes] stores contiguously; host transposes.
"""

import numpy as np

N, K, D, DOUT = 100000, 32, 64, 64
N_CORES = 8
P = 128
SHARD = N // N_CORES            # 12500 nodes per core
TILES = (SHARD + P - 1) // P    # 98
SHARD_PAD = TILES * P           # 12544
NB = 4                          # index banks (int16 gather indices)
BROWS = N // NB                 # 25000 rows per bank
BROWS1 = BROWS + 1              # +1 dummy row at local index 0
G = 4                           # tiles per group
DF = 128                        # padded feature count (gather elem size)
NEG = -1.0e30

TRACE = False                   # test.py sets True to collect HW exec time
LAST_EXEC_TIME_NS = None

_CACHE = {}


# --------------------------------------------------------------------------
# host-side planner
# --------------------------------------------------------------------------

def _groups():
    gs = []
    t = 0
    while t < TILES:
        g = min(G, TILES - t)
        gs.append((t, g))
        t += g
    return gs


def _plan(ei):
    """ei: int32 [N, K] full edge index. Returns the compile-time plan
    (uniform across cores) plus per-core host arrays."""
    groups = _groups()
    n_groups = len(groups)

    bank = (ei // BROWS).astype(np.int32)           # [N, K]
    local = (ei - bank * BROWS + 1).astype(np.int16)  # [N, K], 0 = dummy

    per_core = []
    S_all = np.zeros((n_groups, NB), dtype=np.int64)
    for c in range(N_CORES):
        lo = c * SHARD
        bk = bank[lo:lo + SHARD]                     # [SHARD, K]
        lc = local[lo:lo + SHARD]
        n_ib = np.stack([(bk == b).sum(1) for b in range(NB)], axis=1)

        # permutation: dominant bank, then its count, then 2nd bank/count
        am = n_ib.argmax(1)
        mx = n_ib.max(1)
        am2 = np.argsort(n_ib, axis=1)[:, -2]
        mx2 = np.sort(n_ib, axis=1)[:, -2]
        perm = np.lexsort((mx2, am2, mx, am))        # [SHARD] pos -> node

        # rank of each edge within its (node, bank)
        r = np.zeros((SHARD, K), dtype=np.int32)
        for b in range(NB):
            m = bk == b
            r[m] = (np.cumsum(m, axis=1) - 1)[m]

        pos = np.empty(SHARD, dtype=np.int64)        # node -> sorted pos
        pos[perm] = np.arange(SHARD)

        npad = np.zeros((SHARD_PAD, NB), dtype=np.int64)
        npad[:SHARD] = n_ib[perm]
        Sg = np.zeros((n_groups, NB), dtype=np.int64)
        for gi, (t0, g) in enumerate(groups):
            Sg[gi] = npad[t0 * P:(t0 + g) * P].max(0)
        S_all = np.maximum(S_all, Sg)
        per_core.append(dict(perm=perm, pos=pos, bk=bk, lc=lc, r=r))

    S_all = np.maximum(S_all, 1)                     # keep calls non-empty
    # per-group geometry
    CW = np.array([g * P for (t0, g) in groups])     # node columns
    ST = S_all.sum(1)                                # total slots
    SO = np.cumsum(S_all, axis=1) - S_all            # slot offset per bank
    n_idx = S_all * CW[:, None]                      # num_idxs per (g,b)
    blk_off = np.concatenate([[0], np.cumsum(n_idx.reshape(-1))])
    NI_TOT = int(blk_off[-1]) // 16                  # idx cols per partition
    W_MAX = int((ST * CW).max())                     # gathered cols (worst)

    # per-core int16 index image [128, NI_TOT]
    for c in range(N_CORES):
        pc = per_core[c]
        bk, lc, r, pos = pc["bk"], pc["lc"], pc["r"], pc["pos"]
        flat = np.zeros(int(blk_off[-1]), dtype=np.int16)
        # edge (n, k): group gi = pos[n]//CW.. ; col j = pos[n] - t0*P
        gi_of_pos = np.zeros(SHARD_PAD, dtype=np.int64)
        col_of_pos = np.zeros(SHARD_PAD, dtype=np.int64)
        for gi, (t0, g) in enumerate(groups):
            gi_of_pos[t0 * P:(t0 + g) * P] = gi
            col_of_pos[t0 * P:(t0 + g) * P] = np.arange(g * P)
        pn = pos  # [SHARD]
        e_gi = gi_of_pos[pn][:, None] + np.zeros((1, K), np.int64)
        e_col = col_of_pos[pn][:, None] + np.zeros((1, K), np.int64)
        e_b = bk.astype(np.int64)
        # flat position = blk_off[gi*NB+b] + r*CW[gi] + col
        fpos = (blk_off[(e_gi * NB + e_b).ravel()]
                + r.ravel() * CW[e_gi.ravel()] + e_col.ravel())
        flat[fpos] = lc.ravel()
        # wrap each (g,b) block to [16, n/16] then tile to [128, n/16]
        cols = []
        for gi in range(n_groups):
            for b in range(NB):
                a0, a1 = blk_off[gi * NB + b], blk_off[gi * NB + b + 1]
                blkw = flat[a0:a1].reshape(-1, 16).T     # [16, n/16]
                cols.append(blkw)
        img16 = np.concatenate(cols, axis=1)             # [16, NI_TOT]
        pc["idx_img"] = np.tile(img16, (8, 1))           # [128, NI_TOT]

    return dict(groups=groups, S=S_all, CW=CW, ST=ST, SO=SO,
                n_idx=n_idx, blk_off=blk_off, NI_TOT=NI_TOT, W_MAX=W_MAX,
                per_core=per_core)


# --------------------------------------------------------------------------
# bass kernel
# --------------------------------------------------------------------------

def _build(plan):
    import concourse.bacc as bacc
    import concourse.bass as bass  # noqa: F401
    import concourse.mybir as mybir
    import concourse.tile as tile

    f32 = mybir.dt.float32
    bf16 = mybir.dt.bfloat16
    i16 = mybir.dt.int16

    groups = plan["groups"]
    S, CW, ST, SO = plan["S"], plan["CW"], plan["ST"], plan["SO"]
    n_idx, blk_off, NI_TOT = plan["n_idx"], plan["blk_off"], plan["NI_TOT"]
    W_MAX = plan["W_MAX"]
    CW_MAX = G * P

    nc = bacc.Bacc("TRN2", target_bir_lowering=False, debug=False,
                   num_devices=N_CORES)

    xt_d = nc.dram_tensor("xt", [NB * BROWS1, DF], bf16, kind="ExternalInput")
    xsT_d = nc.dram_tensor("xsT", [DF, SHARD_PAD], bf16, kind="ExternalInput")
    idx_d = nc.dram_tensor("idx", [P, NI_TOT], i16, kind="ExternalInput")
    a_d = nc.dram_tensor("a", [DF, DOUT], bf16, kind="ExternalInput")
    wb_d = nc.dram_tensor("wb", [DF, DOUT], bf16, kind="ExternalInput")
    b_d = nc.dram_tensor("b", [1, DOUT], bf16, kind="ExternalInput")
    outT_d = nc.dram_tensor("outT", [DOUT, SHARD_PAD], f32,
                            kind="ExternalOutput")

    xt_banks = xt_d.ap().rearrange("(b r) d -> b r d", b=NB)

    with tile.TileContext(nc) as tc:
        with (
            tc.tile_pool(name="const", bufs=1) as cpool,
            tc.tile_pool(name="gather", bufs=2) as gpool,
            tc.tile_pool(name="small", bufs=3) as spool,
            tc.tile_pool(name="psum", bufs=2, space="PSUM") as ppool,
        ):
            a_t = cpool.tile([DF, DOUT], bf16)
            nc.sync.dma_start(a_t[:], a_d.ap())
            wb_t = cpool.tile([DF, DOUT], bf16)
            nc.sync.dma_start(wb_t[:], wb_d.ap())
            b_t = cpool.tile([1, DOUT], bf16)
            nc.sync.dma_start(b_t[:], b_d.ap())
            ones1 = cpool.tile([1, CW_MAX], bf16)
            nc.gpsimd.memset(ones1[:], 1.0)

            for gi, (t0, g) in enumerate(groups):
                cw = int(CW[gi])
                st = int(ST[gi])
                w = st * cw

                ni_max = int(max(n_idx[g2].sum() for g2 in range(len(groups)))) // 16
                idxs = spool.tile([P, ni_max], i16, tag="idx")
                io0 = int(blk_off[gi * NB]) // 16
                io1 = int(blk_off[gi * NB + NB]) // 16
                nc.sync.dma_start(idxs[:, :io1 - io0], idx_d.ap()[:, io0:io1])

                xsT_t = spool.tile([DF, CW_MAX], bf16, tag="xsT")
                nc.sync.dma_start(
                    xsT_t[:, :cw], xsT_d.ap()[:, t0 * P:t0 * P + cw])

                gat = gpool.tile([P, W_MAX], bf16, tag="g")
                for b in range(NB):
                    nb = int(n_idx[gi][b])
                    if nb == 0:
                        continue
                    c0 = int(SO[gi][b]) * cw
                    dst = gat[:, c0:c0 + nb].rearrange(
                        "p (one n) -> p one n", one=1)
                    i0 = int(blk_off[gi * NB + b]) // 16 - io0
                    i1 = int(blk_off[gi * NB + b + 1]) // 16 - io0
                    nc.gpsimd.dma_gather(
                        dst, xt_banks[b], idxs[:, i0:i1],
                        nb, nb, DF, transpose=True, single_packet=False,
                    )

                # halving max tree over slots -> M^T at gat[:, :cw]
                stc = st
                while stc > 1:
                    h = (stc + 1) // 2
                    fw = stc - h
                    nc.vector.tensor_tensor(
                        out=gat[:, :fw * cw], in0=gat[:, :fw * cw],
                        in1=gat[:, h * cw:stc * cw],
                        op=mybir.AluOpType.max,
                    )
                    stc = h

                o_p = ppool.tile([DOUT, CW_MAX], f32, tag="o")
                nc.tensor.matmul(o_p[:, :cw], lhsT=a_t[:], rhs=xsT_t[:, :cw],
                                 start=True, stop=False)
                nc.tensor.matmul(o_p[:, :cw], lhsT=wb_t[:], rhs=gat[:, :cw],
                                 start=False, stop=False)
                nc.tensor.matmul(o_p[:, :cw], lhsT=b_t[:], rhs=ones1[:, :cw],
                                 start=False, stop=True)

                o_s = spool.tile([DOUT, CW_MAX], f32, tag="os")
                nc.scalar.copy(out=o_s[:, :cw], in_=o_p[:, :cw])
                nc.sync.dma_start(
                    outT_d.ap()[:, t0 * P:t0 * P + cw], o_s[:, :cw])

    nc.compile()
    return nc


# --------------------------------------------------------------------------
# host wrapper
# --------------------------------------------------------------------------

def _prep_inputs(x, W, b, plan):
    import ml_dtypes
    bf16 = ml_dtypes.bfloat16

    x = np.asarray(x, dtype=np.float32)
    W = np.asarray(W, dtype=np.float32)
    b = np.asarray(b, dtype=np.float32).reshape(1, DOUT)

    # banked, feature-padded bf16 table with dummy row 0 per bank
    xt = np.zeros((NB * BROWS1, DF), dtype=bf16)
    xtv = xt.reshape(NB, BROWS1, DF)
    xtv[:, 0, :] = bf16(NEG)
    xtv[:, 1:, :D] = x.reshape(NB, BROWS, D).astype(bf16)

    A = (W[:D] - W[D:]).astype(bf16)
    Wb = W[D:].astype(bf16)
    a_pad = np.zeros((DF, DOUT), dtype=bf16)
    a_pad[:D] = A
    wb_pad = np.zeros((DF, DOUT), dtype=bf16)
    wb_pad[:D] = Wb
    b_bf = b.astype(bf16)

    in_maps = []
    for c in range(N_CORES):
        pc = plan["per_core"][c]
        perm = pc["perm"]
        xs = np.zeros((SHARD_PAD, D), np.float32)
        xs[:SHARD] = x[c * SHARD:(c + 1) * SHARD][perm]
        xsT = np.zeros((DF, SHARD_PAD), dtype=bf16)
        xsT[:D] = xs.T.astype(bf16)
        in_maps.append({
            "xt": xt, "xsT": xsT, "idx": pc["idx_img"],
            "a": a_pad, "wb": wb_pad, "b": b_bf,
        })
    return in_maps


def _install_trace_shim():
    """Provide antenv.axon_hooks (missing in this image) so
    run_bass_kernel_spmd(trace=True) can collect an NTFF profile."""
    import sys
    import types
    try:
        from antenv import axon_hooks  # noqa: F401
        return
    except ImportError:
        pass
    import antenv
    from concourse import bass_utils
    mod = types.ModuleType("antenv.axon_hooks")
    _hook = [None]
    mod.set_axon_ntff_profile_hook = lambda h: _hook.__setitem__(0, h)
    mod.get_axon_ntff_profile_hook = lambda: _hook[0]
    sys.modules["antenv.axon_hooks"] = mod
    antenv.axon_hooks = mod
    from trn_agent_boot.trn_boot import _ntff_profile_via_ctypes
    mod.set_axon_ntff_profile_hook(
        _ntff_profile_via_ctypes("/opt/axon/libaxon_pjrt.so"))
    bass_utils.upload_artifacts = lambda d: d


def kernel(x, edge_index, W, b):
    global LAST_EXEC_TIME_NS
    from concourse import bass_utils

    if TRACE:
        _install_trace_shim()

    ei = np.asarray(edge_index).astype(np.int32)
    key = hash(ei.tobytes())
    if _CACHE.get("key") != key:
        plan = _plan(ei)
        _CACHE.clear()
        _CACHE.update(key=key, plan=plan, nc=_build(plan))
    plan, nc = _CACHE["plan"], _CACHE["nc"]

    in_maps = _prep_inputs(x, W, b, plan)
    res = bass_utils.run_bass_kernel_spmd(
        nc, in_maps, core_ids=list(range(N_CORES)), trace=TRACE,
    )
    LAST_EXEC_TIME_NS = res.exec_time_ns

    out = np.empty((N, DOUT), np.float32)
    for c in range(N_CORES):
        resT = np.asarray(res.results[c]["outT"], dtype=np.float32).T
        perm = plan["per_core"][c]["perm"]
        oc = out[c * SHARD:(c + 1) * SHARD]
        oc[perm] = resT[:SHARD]
    return out


# --------------------------------------------------------------------------
# host-side numpy validation of the plan (no HW)
# --------------------------------------------------------------------------

def _selfcheck_plan(x, ei, plan):
    import ml_dtypes
    bf16 = ml_dtypes.bfloat16
    xb = np.zeros((NB, BROWS1, DF), np.float32)
    xb[:, 0, :] = NEG
    xb[:, 1:, :D] = np.asarray(x, np.float32).reshape(NB, BROWS, D) \
        .astype(bf16).astype(np.float32)
    groups = plan["groups"]
    S, CW, SO, blk_off = plan["S"], plan["CW"], plan["SO"], plan["blk_off"]
    rng = np.random.default_rng(0)
    for c in range(N_CORES):
        pc = plan["per_core"][c]
        img = pc["idx_img"][:16]        # [16, NI]
        perm = pc["perm"]
        for gi in rng.choice(len(groups), 4, replace=False):
            t0, g = groups[gi]
            cw = int(CW[gi])
            st = int(plan["ST"][gi])
            # reconstruct gathered buffer [DF, st*cw]
            buf = np.empty((DF, st * cw), np.float32)
            for b in range(NB):
                a0, a1 = blk_off[gi * NB + b], blk_off[gi * NB + b + 1]
                n = int(a1 - a0)
                if n == 0:
                    continue
                flat = img[:, a0 // 16:a1 // 16].T.reshape(-1)  # positions
                vals = xb[b][flat.astype(np.int64)]             # [n, DF]
                c0 = int(SO[gi][b]) * cw
                buf[:, c0:c0 + n] = vals.T
            M_T = buf.reshape(DF, st, cw).max(1)                # [DF, cw]
            # expected
            pos_ids = np.arange(t0 * P, t0 * P + cw)
            nodes = np.where(pos_ids < SHARD,
                             perm[np.minimum(pos_ids, SHARD - 1)], -1)
            eic = np.asarray(ei[c * SHARD:(c + 1) * SHARD], np.int64)
            xf = np.asarray(x, np.float32).astype(bf16).astype(np.float32)
            for j in rng.choice(cw, 8, replace=False):
                nd = nodes[j]
                if nd < 0:
                    continue
                exp = xf[eic[nd]].max(0)
                got = M_T[:D, j]
                assert np.allclose(exp, got, atol=1e-2), (c, gi, j, exp[:4], got[:4])
    print("plan selfcheck OK")


if __name__ == "__main__":
    rng = np.random.default_rng(0)
    x = rng.standard_normal((N, D), dtype=np.float32)
    ei = rng.integers(0, N, (N, K)).astype(np.int64)
    W = (rng.standard_normal((2 * D, DOUT)) / np.sqrt(2 * D)).astype(np.float32)
    b = np.zeros(DOUT, np.float32)
    plan = _plan(ei.astype(np.int32))
    print("W_MAX cols:", plan["W_MAX"], "NI_TOT:", plan["NI_TOT"],
          "slot ratio:", (plan["ST"] * plan["CW"]).sum() / (TILES * P * K))
    _selfcheck_plan(x, ei.astype(np.int32), plan)
    out = kernel(x, ei, W, b)
    M = np.max(x[ei], axis=1)
    exp = x @ (W[:D] - W[D:]) + M @ W[D:] + b
    err = np.abs(out - exp).max() / np.abs(exp).max()
    print("rel err:", err)
